# revision 21
# baseline (speedup 1.0000x reference)
"""AGNN (2-layer) distributed Bass kernel for one TRN2 chip (8 NeuronCores).

v2 design. Nodes row-block-sharded across 8 cores (6250 each); each core owns
edges whose destination is local. Per AGNN layer, per core:
  - builds a 256B table row per local node: [xn(64) | rcpn | norm | pad] bf16
    (256B is the dma_gather minimum row size)
  - the table is split A/B by local row (A = rows 0:4096, B = 4096:6250) so
    src-gather indices fit int16 after AllGather concatenation, and so the
    pass-A edge processing overlaps the table-B AllGather
  - gathers per-edge SRC rows only (dma_gather, SEG chunks per call).
    DST features are NOT gathered: per 128-edge chunk a small PE matmul
    dst_feat[e,f] = sum_w MT[w,e] * wtab[w,f] materializes them from the
    window's 64 local table rows (wtab, SBUF) using a host-precomputed
    transposed one-hot mask MT streamed via plain contiguous DMA
  - per-edge dots on DVE against the dst_feat PSUM (8 chunks per PSUM bank),
    tree-reduced; w = exp(beta*a); wn = w*norm_src
  - scatter-add via one-hot matmuls: lhsT = M (host-streamed one-hot, scaled
    by wn), rhs = [xn_src | rcpn_src] -> psum [64, 65] per window gives
    numerator and denominator simultaneously
lin1/lin2 GEMMs are node-parallel (x shipped pre-transposed bf16 in
partition-contiguous layout); log_softmax fused at the end.
"""

import contextlib
import numpy as np
import ml_dtypes

from concourse import bacc, bass, mybir, tile

BF16 = mybir.dt.bfloat16
F32 = mybir.dt.float32
I16 = mybir.dt.int16
NPBF = ml_dtypes.bfloat16

EPS = 1e-12
# TimelineSim cost-model total is 1,335,398 ns for this config; the same
# simulator scores the previous 752,900 ns HW baseline at 1,479,300 ns, so
# the expected HW time is ~1335 * (752.9/1479.3) ~= 680 us.  Engine busy in
# the sim: DMA 650us (gathers 333 + mask streams/repacks), DVE 410us,
# collectives 390us (4x packed-row AllGathers), Pool 340us, PE 186us.
LAST_COST_MODEL_NS = 680_000
ROW = 128          # table row width (bf16 elements) = 256 bytes (HW minimum)
RCPN, NORM = 64, 65   # column layout within a table row


class Cfg:
    def __init__(self, N=50000, E=800000, F_in=1024, H=64, C=256, P=8,
                 WIN=64, CHUNK=128, WAVE_WINDOWS=6, SEG=8, SPLA=4096,
                 debug_layers=2):
        self.N, self.E, self.F_in, self.H, self.C, self.P = N, E, F_in, H, C, P
        self.WIN, self.CHUNK = WIN, CHUNK
        self.WAVE_WINDOWS = WAVE_WINDOWS
        self.SEG = SEG
        self.debug_layers = debug_layers
        assert N % P == 0
        self.n_loc = N // P
        self.SPLA = SPLA                      # local rows in table A
        self.SPLB = self.n_loc - SPLA         # local rows in table B
        assert SPLA % 128 == 0
        self.n_rtA = SPLA // 128
        assert P * SPLA <= 32768 and P * self.SPLB <= 32768
        self.n_win = (self.n_loc + WIN - 1) // WIN
        self.n_rt = (self.n_loc + 127) // 128
        self.rt_tail = self.n_loc - (self.n_rt - 1) * 128
        assert F_in % 128 == 0
        self.n_kc = F_in // 128


# ----------------------------------------------------------------------------
# host-side edge prep
# ----------------------------------------------------------------------------
def host_prep(cfg, edge_index):
    src = np.asarray(edge_index[0], dtype=np.int64)
    dst = np.asarray(edge_index[1], dtype=np.int64)
    P, n_loc, WIN, CHUNK, n_win = cfg.P, cfg.n_loc, cfg.WIN, cfg.CHUNK, cfg.n_win
    SPLA, SPLB = cfg.SPLA, cfg.SPLB

    core_of = dst // n_loc
    win_of = (dst % n_loc) // WIN
    lr_src = src % n_loc
    core_src = src // n_loc
    b_of = (lr_src >= SPLA).astype(np.int64)
    tab_idx = np.where(b_of, core_src * SPLB + (lr_src - SPLA),
                       core_src * SPLA + lr_src)
    key = (core_of * n_win + win_of) * 2 + b_of
    order = np.argsort(key, kind="stable")
    bounds = np.searchsorted(key[order], np.arange(P * n_win * 2 + 1))

    def bucket_len(c, w, h):
        i = (c * n_win + w) * 2 + h
        return bounds[i + 1] - bounds[i]

    def bucket(c, w, h):
        i = (c * n_win + w) * 2 + h
        return order[bounds[i]:bounds[i + 1]]

    K_A = np.zeros(n_win, dtype=np.int64)
    K_B = np.zeros(n_win, dtype=np.int64)
    for w in range(n_win):
        mA = max(bucket_len(c, w, 0) for c in range(P))
        mB = max(bucket_len(c, w, 1) for c in range(P))
        K_A[w] = max(1, -(-mA // CHUNK))
        K_B[w] = max(1, -(-mB // CHUNK))
    cA_total = int(K_A.sum())
    c_total = cA_total + int(K_B.sum())

    win_of_chunk = np.zeros(c_total, dtype=np.int64)
    start_flag = np.zeros(c_total, dtype=bool)
    stop_flag = np.zeros(c_total, dtype=bool)

    # per-core slot arrays
    slots_tab = np.zeros((P, c_total * 128), dtype=np.int64)   # table row idx
    slots_rel = np.full((P, c_total * 128), -1.0, dtype=np.float32)

    # waves: two sequences (pass A over K_A chunks, pass B over K_B chunks)
    waves = []
    j0 = 0
    for h, K in ((0, K_A), (1, K_B)):
        for w0 in range(0, n_win, cfg.WAVE_WINDOWS):
            ws = list(range(w0, min(w0 + cfg.WAVE_WINDOWS, n_win)))
            Wc = int(sum(K[w] for w in ws))
            col = j0
            for w in ws:
                start_flag[col] = True
                for c in range(P):
                    el = bucket(c, w, h)
                    k = np.arange(len(el))
                    pos = (col + k // CHUNK) * 128 + k % CHUNK
                    slots_tab[c, pos] = tab_idx[el]
                    slots_rel[c, pos] = (dst[el] % n_loc) - w * WIN
                win_of_chunk[col:col + K[w]] = w
                col += int(K[w])
                stop_flag[col - 1] = True
            waves.append(dict(j0=j0, Wc=Wc, n_wins=len(ws), w_base=w0,
                              is_b=h))
            j0 += Wc
    assert j0 == c_total
    n_waves_a = sum(1 for v in waves if v["is_b"] == 0)

    # wrapped int16 idx layout (position-uniform: slot i -> col i//16 with
    # 8 replicated 16-partition groups)
    idx_src_w = np.zeros((P, 128, c_total * 8), dtype=np.int16)
    i = np.arange(c_total * 128)
    cols = i // 16
    rows = i % 16
    for c in range(P):
        v = slots_tab[c].astype(np.int16)
        for g in range(8):
            idx_src_w[c, g * 16 + rows, cols] = v

    # masks, bf16: M per-wave w-major [128, WIN*Wc] blocks (so the wn scale
    # multiply has packed innermost dims -> DVE 2x); MT [WIN, c_total*128]
    rel = slots_rel.reshape(P, c_total, 128)
    iota = np.arange(WIN, dtype=np.float32)
    m_host = np.zeros((P, 128, c_total * WIN), dtype=NPBF)
    mt_host = np.zeros((P, WIN, c_total * 128), dtype=NPBF)
    for c in range(P):
        mm = (rel[c][:, :, None] == iota[None, None, :])      # [ct, 128, WIN]
        mt_host[c] = np.ascontiguousarray(
            mm.transpose(2, 0, 1).reshape(WIN, c_total * 128)).astype(NPBF)
        for v in waves:
            j0, Wc = v["j0"], v["Wc"]
            blk = mm[j0:j0 + Wc].transpose(1, 2, 0)           # [128, WIN, Wc]
            m_host[c][:, j0 * WIN:(j0 + Wc) * WIN] = np.ascontiguousarray(
                blk.reshape(128, WIN * Wc)).astype(NPBF)

    wc_max = max(v["Wc"] for v in waves)
    return dict(
        idx_src_w=idx_src_w,
        m_host=m_host,
        mt_host=mt_host,
        waves=waves,
        n_waves_a=n_waves_a,
        c_total=c_total,
        cA_total=cA_total,
        wc_max=wc_max,
        win_of_chunk=win_of_chunk,
        start_flag=start_flag,
        stop_flag=stop_flag,
        dbg_slots_tab=slots_tab,
        dbg_slots_rel=slots_rel,
    )


# ----------------------------------------------------------------------------
# numpy emulator of the device algorithm (host-prep validation)
# ----------------------------------------------------------------------------
def emulate(cfg, prep, inputs):
    x = np.asarray(inputs["x"], dtype=np.float64)
    w1 = np.asarray(inputs["lin1_w"], dtype=np.float64)
    b1 = np.asarray(inputs["lin1_b"], dtype=np.float64)
    w2 = np.asarray(inputs["lin2_w"], dtype=np.float64)
    b2 = np.asarray(inputs["lin2_b"], dtype=np.float64)
    betas = [float(inputs["beta1"][0]), float(inputs["beta2"][0])]
    P, n_loc, WIN = cfg.P, cfg.n_loc, cfg.WIN
    SPLA, SPLB = cfg.SPLA, cfg.SPLB
    c_total, cA = prep["c_total"], prep["cA_total"]
    win_of_chunk = prep["win_of_chunk"]
    slots_tab = prep["dbg_slots_tab"]
    slots_rel = prep["dbg_slots_rel"]

    h = np.maximum(x @ w1.T + b1, 0.0)
    for layer in range(cfg.debug_layers):
        beta = betas[layer]
        nrm = np.maximum(np.linalg.norm(h, axis=1), EPS)
        xn = h / nrm[:, None]
        h2 = np.zeros_like(h)
        for c in range(P):
            num = np.zeros((n_loc, cfg.H))
            den = np.zeros(n_loc)
            for j in range(c_total):
                is_b = j >= cA
                w = win_of_chunk[j]
                sl = slice(j * 128, (j + 1) * 128)
                rel = slots_rel[c, sl]
                valid = rel >= 0
                ti = slots_tab[c, sl][valid]
                if is_b:
                    s_node = (ti // SPLB) * n_loc + SPLA + ti % SPLB
                else:
                    s_node = (ti // SPLA) * n_loc + ti % SPLA
                d_loc = (w * WIN + rel[valid]).astype(np.int64)
                a = np.sum(xn[s_node] * xn[c * n_loc + d_loc], axis=1)
                wgt = np.exp(beta * a)
                np.add.at(num, d_loc, wgt[:, None] * h[s_node])
                np.add.at(den, d_loc, wgt)
            h2[c * n_loc:(c + 1) * n_loc] = num / np.maximum(den, EPS)[:, None]
        h = h2
    logits = h @ w2.T + b2
    lse = np.log(np.sum(np.exp(logits - logits.max(axis=1, keepdims=True)),
                        axis=1)) + logits.max(axis=1)
    return logits - lse[:, None]


# ----------------------------------------------------------------------------
# device program
# ----------------------------------------------------------------------------
def build_program(cfg, prep):
    P, H, C, F_in = cfg.P, cfg.H, cfg.C, cfg.F_in
    n_loc, n_rt, rt_tail, n_kc = cfg.n_loc, cfg.n_rt, cfg.rt_tail, cfg.n_kc
    WIN, N, SPLA, SPLB = cfg.WIN, cfg.N, cfg.SPLA, cfg.SPLB
    n_rtA, SEG = cfg.n_rtA, cfg.SEG
    c_total, waves, wc_max = prep["c_total"], prep["waves"], prep["wc_max"]
    win_of_chunk = prep["win_of_chunk"]
    start_flag, stop_flag = prep["start_flag"], prep["stop_flag"]
    n_win = cfg.n_win

    nc = bacc.Bacc("TRN2", target_bir_lowering=False,
                   dynamic_dma_scratch_size=16 * 128 * SEG)

    xTp_ext = nc.declare_dram_parameter("xTp", [128, n_rt * 8 * 128], BF16,
                                        isOutput=False)
    w1t_ext = nc.declare_dram_parameter("w1t", [128, n_kc * H], BF16,
                                        isOutput=False)
    b1r_ext = nc.declare_dram_parameter("b1r", [128, H], F32, isOutput=False)
    w2t_ext = nc.declare_dram_parameter("w2t", [H, C], BF16, isOutput=False)
    b2r_ext = nc.declare_dram_parameter("b2r", [128, C], BF16, isOutput=False)
    beta1_ext = nc.declare_dram_parameter("beta1r", [128, 1], F32, isOutput=False)
    beta2_ext = nc.declare_dram_parameter("beta2r", [128, 1], F32, isOutput=False)
    isrc_ext = nc.declare_dram_parameter(
        "idx_src_w", [128, c_total * 8], I16, isOutput=False)
    m_ext = nc.declare_dram_parameter(
        "m_host", [128, c_total * WIN], BF16, isOutput=False)
    mt_ext = nc.declare_dram_parameter(
        "mt_host", [WIN, c_total * 128], BF16, isOutput=False)
    ident_ext = nc.declare_dram_parameter("ident", [128, 128], BF16,
                                          isOutput=False)
    out_ext = nc.declare_dram_parameter("out", [n_loc, C], F32, isOutput=True)

    PK = NORM + 1   # packed row width (132B) for collective transport
    loc_packA = [nc.dram_tensor(f"loc_packA{i}", [SPLA, PK], BF16) for i in (0, 1)]
    loc_packB = [nc.dram_tensor(f"loc_packB{i}", [SPLB, PK], BF16) for i in (0, 1)]
    full_packA = [
        nc.dram_tensor(f"full_packA{i}", [P * SPLA, PK], BF16, addr_space="Shared")
        for i in (0, 1)
    ]
    full_packB = [
        nc.dram_tensor(f"full_packB{i}", [P * SPLB, PK], BF16, addr_space="Shared")
        for i in (0, 1)
    ]
    full_tabA = [nc.dram_tensor(f"full_tabA{i}", [P * SPLA, ROW], BF16)
                 for i in (0, 1)]
    full_tabB = [nc.dram_tensor(f"full_tabB{i}", [P * SPLB, ROW], BF16)
                 for i in (0, 1)]
    replica = [list(range(P))]

    with contextlib.ExitStack() as es:
        tc = es.enter_context(tile.TileContext(nc))
        const = es.enter_context(tc.tile_pool(name="const", bufs=1))
        sb = es.enter_context(tc.tile_pool(name="sb", bufs=1))

        # ------- constants -------
        w1t_b = const.tile([128, n_kc, H], BF16)
        nc.sync.dma_start(out=w1t_b[:], in_=w1t_ext[:].rearrange(
            "p (k h) -> p k h", k=n_kc))
        b1r = const.tile([128, H], F32)
        nc.sync.dma_start(out=b1r[:], in_=b1r_ext[:])
        b2r_b = const.tile([128, C], BF16)
        nc.sync.dma_start(out=b2r_b[:], in_=b2r_ext[:])
        w2t_b = const.tile([H, C], BF16)
        nc.sync.dma_start(out=w2t_b[:], in_=w2t_ext[:])
        beta = []
        for i, ext in enumerate((beta1_ext, beta2_ext)):
            bt = const.tile([128, 1], F32, name=f"beta{i}")
            nc.sync.dma_start(out=bt[:], in_=ext[:])
            beta.append(bt)
        ident = const.tile([128, 128], BF16)
        nc.sync.dma_start(out=ident[:], in_=ident_ext[:])
        isrc = const.tile([128, c_total * 8], I16)
        nc.sync.dma_start(out=isrc[:], in_=isrc_ext[:])

        h_loc = sb.tile([128, n_rt, H], BF16)
        nsq = sb.tile([128, n_rt], F32)
        nc.vector.memset(h_loc[:, n_rt - 1, :], 0.0)

        # ------- phase A: lin1 + relu (+ per-tile nsq) -------
        with tc.tile_pool(name="ph_a", bufs=3) as pa, \
             tc.tile_pool(name="ph_a_ps", bufs=2, space="PSUM") as pa_ps:
            for r in range(n_rt):
                rows = 128 if r < n_rt - 1 else rt_tail
                xt_b = pa.tile([128, n_kc, 128], BF16, tag="xt_b")
                nc.sync.dma_start(
                    out=xt_b[:],
                    in_=xTp_ext[:, r * 1024:(r + 1) * 1024].rearrange(
                        "p (k n) -> p k n", k=n_kc),
                )
                hp = pa_ps.tile([128, H], F32, tag="hp")
                for k in range(n_kc):
                    nc.tensor.matmul(
                        hp[:rows, :],
                        lhsT=xt_b[:, k, :rows],
                        rhs=w1t_b[:, k, :],
                        start=(k == 0),
                        stop=(k == n_kc - 1),
                    )
                hb = pa.tile([128, H], F32, tag="hb")
                nc.vector.tensor_tensor(
                    out=hb[:rows, :], in0=hp[:rows, :], in1=b1r[:rows, :],
                    op=mybir.AluOpType.add,
                )
                nc.vector.tensor_scalar(
                    out=h_loc[:rows, r, :], in0=hb[:rows, :],
                    scalar1=0.0, scalar2=None, op0=mybir.AluOpType.max,
                )
                sq = pa.tile([128, H], BF16, tag="sq")
                nc.scalar.activation(
                    out=sq[:], in_=h_loc[:, r, :],
                    func=mybir.ActivationFunctionType.Square,
                    accum_out=nsq[:, r:r + 1],
                )

        # ------- shared per-layer tiles -------
        tabrow = sb.tile([128, n_rt, ROW], BF16)
        wtab = sb.tile([WIN, n_win, WIN], BF16)
        aggA = sb.tile([128, n_rt, H], BF16)
        aggB = sb.tile([128, n_rt, H], BF16)
        denA = sb.tile([128, n_rt], F32)
        denB = sb.tile([128, n_rt], F32)

        # ------- table build + allgather (split A/B by local row) -------
        def build_tables(src_tile, nsq_tile, layer):
            norm = sb.tile([128, n_rt], F32, name=f"norm{layer}")
            rcpn = sb.tile([128, n_rt], F32, name=f"rcpn{layer}")
            nc.scalar.sqrt(norm[:], nsq_tile[:])
            nc.vector.tensor_scalar(
                out=norm[:], in0=norm[:], scalar1=float(EPS), scalar2=None,
                op0=mybir.AluOpType.max,
            )
            nc.vector.reciprocal(rcpn[:], norm[:])

            def fill_rows(r0, r1):
                nc.vector.memset(tabrow[:, r0:r1, :], 0.0)
                for r in range(r0, r1):
                    nc.vector.tensor_scalar(
                        out=tabrow[:, r, 0:H], in0=src_tile[:, r, :],
                        scalar1=rcpn[:, r:r + 1], scalar2=None,
                        op0=mybir.AluOpType.mult,
                    )
                nc.vector.tensor_copy(tabrow[:, r0:r1, RCPN], rcpn[:, r0:r1])
                nc.vector.tensor_copy(tabrow[:, r0:r1, NORM], norm[:, r0:r1])

            # part A: r-tiles [0, n_rtA).  loc writes ride the gpsimd (Pool)
            # queue so their waits never block the SP stream queue; repacks
            # are emitted inside agnn_layer just before the pass that needs
            # them (again on gpsimd, ahead of that pass's gathers).
            fill_rows(0, n_rtA)
            ltA = loc_packA[layer]
            nc.sync.dma_start(
                out=ltA[:].rearrange("(t p) f -> p t f", p=128),
                in_=tabrow[:, 0:n_rtA, 0:PK],
            )
            nc.gpsimd.collective_compute(
                "AllGather", mybir.AluOpType.bypass, replica_groups=replica,
                ins=[ltA[:].opt()], outs=[full_packA[layer][:].opt()],
            )
            # part B: r-tiles [n_rtA, n_rt)
            fill_rows(n_rtA, n_rt)
            ltB = loc_packB[layer]
            nfullB = (n_rt - 1 - n_rtA) * 128
            if nfullB > 0:
                nc.sync.dma_start(
                    out=ltB[0:nfullB, :].rearrange("(t p) f -> p t f", p=128),
                    in_=tabrow[:, n_rtA:n_rt - 1, 0:PK],
                )
            nc.sync.dma_start(
                out=ltB[nfullB:SPLB, :], in_=tabrow[0:rt_tail, n_rt - 1, 0:PK]
            )
            nc.gpsimd.collective_compute(
                "AllGather", mybir.AluOpType.bypass, replica_groups=replica,
                ins=[ltB[:].opt()], outs=[full_packB[layer][:].opt()],
            )
            # window-major dst table (xn only): even/odd windows
            nc.sync.dma_start(out=wtab[:, 0:n_win:2, :],
                              in_=tabrow[0:WIN, :, 0:WIN])
            nc.sync.dma_start(out=wtab[:, 1:n_win:2, :],
                              in_=tabrow[WIN:128, :, 0:WIN])

        # ------- one AGNN layer -------
        def agnn_layer(layer, beta_tile, agg_out, nsq_out, work, psum_s,
                       psum_d):
            for agg_t, den_t, is_b in ((aggA, denA, 0), (aggB, denB, 1)):
                ft = (full_tabB if is_b else full_tabA)[layer]
                fp = (full_packB if is_b else full_packA)[layer]
                # repack 132B collective rows to the 256B-stride gather table;
                # on the gpsimd queue right ahead of this pass's gathers
                # (halved: DMA APs must stay under 32768 descriptors)
                nrows = fp.shape[0]
                nc.scalar.dma_start(out=ft[0:nrows // 2, 0:PK],
                                    in_=fp[0:nrows // 2, :])
                nc.scalar.dma_start(out=ft[nrows // 2:nrows, 0:PK],
                                    in_=fp[nrows // 2:nrows, :])
                for g, wave in enumerate(waves):
                    if wave["is_b"] != is_b:
                        continue
                    Wc, j0, w_base = wave["Wc"], wave["j0"], wave["w_base"]
                    gsrc = work.tile([128, wc_max, ROW], BF16, tag="gsrc")
                    for s0 in range(0, Wc, SEG):
                        sn = min(SEG, Wc - s0)
                        nc.gpsimd.dma_gather(
                            out_ap=gsrc[:, s0:s0 + sn, :],
                            in_ap=ft[:, :],
                            idxs_ap=isrc[:, (j0 + s0) * 8:(j0 + s0 + sn) * 8],
                            num_idxs=sn * 128, num_idxs_reg=sn * 128,
                            elem_size=ROW,
                        )
                    mt_t = work.tile([WIN, wc_max, 128], BF16, tag="mt")
                    nc.sync.dma_start(
                        out=mt_t[:, 0:Wc, :],
                        in_=mt_ext[:, j0 * 128:(j0 + Wc) * 128].rearrange(
                            "w (c e) -> w c e", c=Wc),
                    )
                    m_t = work.tile([128, wc_max * WIN], BF16, tag="m")
                    nc.sync.dma_start(
                        out=m_t[:, 0:Wc * WIN],
                        in_=m_ext[:, j0 * WIN:(j0 + Wc) * WIN],
                    )
                    dotb = work.tile([128, wc_max, WIN], BF16, tag="dot")
                    for g0 in range(0, Wc, 8):
                        gn = min(8, Wc - g0)
                        psd = psum_d.tile([128, 8, WIN], F32, tag="psd",
                                          name=f"psd_{layer}_{g}_{g0}")
                        for k in range(gn):
                            j = j0 + g0 + k
                            nc.tensor.matmul(
                                psd[:, k, :],
                                lhsT=mt_t[:, g0 + k, :],
                                rhs=wtab[:, int(win_of_chunk[j]), :],
                                start=True, stop=True,
                            )
                        nc.vector.tensor_tensor(
                            out=dotb[:, g0:g0 + gn, :],
                            in0=gsrc[:, g0:g0 + gn, 0:H],
                            in1=psd[:, 0:gn, :],
                            op=mybir.AluOpType.mult,
                        )
                    width = WIN
                    while width > 2:
                        half = width // 2
                        nc.vector.tensor_tensor(
                            out=dotb[:, 0:Wc, 0:half],
                            in0=dotb[:, 0:Wc, 0:half],
                            in1=dotb[:, 0:Wc, half:width],
                            op=mybir.AluOpType.add,
                        )
                        width = half
                    a_t = work.tile([128, wc_max], BF16, tag="a_t")
                    nc.vector.tensor_tensor(
                        out=a_t[:, 0:Wc], in0=dotb[:, 0:Wc, 0],
                        in1=dotb[:, 0:Wc, 1], op=mybir.AluOpType.add,
                    )
                    w_t = work.tile([128, wc_max], BF16, tag="w_t")
                    nc.scalar.activation(
                        out=w_t[:, 0:Wc], in_=a_t[:, 0:Wc],
                        func=mybir.ActivationFunctionType.Exp,
                        scale=beta_tile[:, 0:1],
                    )
                    wn_t = work.tile([128, wc_max], BF16, tag="wn_t")
                    nc.vector.tensor_tensor(
                        out=wn_t[:, 0:Wc], in0=w_t[:, 0:Wc],
                        in1=gsrc[:, 0:Wc, NORM], op=mybir.AluOpType.mult,
                    )
                    # m is w-major per wave: [128, WIN, Wc]; wn broadcasts
                    # along w with packed innermost c -> DVE 2x mode
                    m_v = m_t[:, 0:Wc * WIN].rearrange("p (w c) -> p w c", c=Wc)
                    nc.vector.tensor_tensor(
                        out=m_v, in0=m_v,
                        in1=wn_t[:, None, 0:Wc].broadcast_to([128, WIN, Wc]),
                        op=mybir.AluOpType.mult,
                    )
                    pslots = [
                        psum_s.tile([WIN, H + 1], F32, tag=f"ps{s}",
                                    name=f"ps{s}_{layer}_{g}")
                        for s in range(wave["n_wins"])
                    ]
                    for k in range(Wc):
                        j = j0 + k
                        s = int(win_of_chunk[j] - w_base)
                        nc.tensor.matmul(
                            pslots[s][:, :],
                            lhsT=m_t[:, k:k + (WIN - 1) * Wc + 1:Wc],
                            rhs=gsrc[:, k, 0:H + 1],
                            start=bool(start_flag[j]),
                            stop=bool(stop_flag[j]),
                        )
                    for s in range(wave["n_wins"]):
                        w = w_base + s
                        prow = (w % 2) * WIN
                        nc.scalar.copy(
                            agg_t[prow:prow + WIN, w // 2, :],
                            pslots[s][:, 0:H],
                        )
                        nc.vector.tensor_copy(
                            den_t[prow:prow + WIN, w // 2:w // 2 + 1],
                            pslots[s][:, H:H + 1],
                        )
            # combine A+B, divide
            den_r = sb.tile([128, n_rt], F32, name=f"den{layer}")
            nc.vector.tensor_tensor(out=den_r[:], in0=denA[:], in1=denB[:],
                                    op=mybir.AluOpType.add)
            nc.vector.tensor_scalar(
                out=den_r[:], in0=den_r[:], scalar1=float(EPS), scalar2=None,
                op0=mybir.AluOpType.max,
            )
            rcpden = sb.tile([128, n_rt], F32, name=f"rcpden{layer}")
            nc.vector.reciprocal(rcpden[:], den_r[:])
            rcpden_b = sb.tile([128, n_rt], BF16, name=f"rcpdenb{layer}")
            nc.vector.tensor_copy(rcpden_b[:], rcpden[:])
            num = sb.tile([128, n_rt, H], BF16, name=f"num{layer}")
            nc.vector.tensor_tensor(out=num[:], in0=aggA[:], in1=aggB[:],
                                    op=mybir.AluOpType.add)
            nc.vector.tensor_tensor(
                out=agg_out[:, :, :], in0=num[:, :, :],
                in1=rcpden_b[:, :].to_broadcast([128, n_rt, H]),
                op=mybir.AluOpType.mult,
            )
            sqscr = sb.tile([128, n_rt, H], BF16, name=f"sq{layer}")
            nc.vector.tensor_tensor(
                out=sqscr[:], in0=agg_out[:], in1=agg_out[:],
                op=mybir.AluOpType.mult,
            )
            nc.vector.tensor_reduce(
                out=nsq_out[:], in_=sqscr[:], axis=mybir.AxisListType.X,
                op=mybir.AluOpType.add,
            )

        x1 = sb.tile([128, n_rt, H], BF16)
        nsq1 = sb.tile([128, n_rt], F32)
        x2 = sb.tile([128, n_rt, H], BF16)
        nsq2 = sb.tile([128, n_rt], F32)
        with tc.tile_pool(name="work", bufs=2) as work, \
             tc.tile_pool(name="psum_s", bufs=1, space="PSUM") as psum_s, \
             tc.tile_pool(name="psum_d", bufs=2, space="PSUM") as psum_d:
            if cfg.debug_layers >= 1:
                build_tables(h_loc, nsq, layer=0)
                agnn_layer(0, beta[0], x1, nsq1, work, psum_s, psum_d)
            else:
                nc.vector.memset(x1[:], 0.0)
                nc.vector.memset(nsq1[:], 1.0)
            if cfg.debug_layers >= 2:
                build_tables(x1, nsq1, layer=1)
                agnn_layer(1, beta[1], x2, nsq2, work, psum_s, psum_d)
            else:
                nc.vector.memset(x2[:], 0.0)

        # ------- phase C: lin2 + log_softmax -------
        with tc.tile_pool(name="ph_c", bufs=2) as pc, \
             tc.tile_pool(name="ph_c1", bufs=1) as pc1, \
             tc.tile_pool(name="ph_c_ps", bufs=2, space="PSUM") as pc_ps:
            logits = pc1.tile([128, n_rt, C], BF16)
            for r in range(n_rt):
                tp = pc_ps.tile([H, 128], BF16, tag="tp")
                nc.tensor.transpose(tp[:, :], x2[:, r, :], ident[:])
                x2t = pc.tile([H, 128], BF16, tag="x2t")
                nc.scalar.copy(x2t[:], tp[:])
                lp = pc_ps.tile([128, C], F32, tag="lp")
                nc.tensor.matmul(lp[:], lhsT=x2t[:], rhs=w2t_b[:],
                                 start=True, stop=True)
                nc.vector.scalar_tensor_tensor(
                    out=logits[:, r, :], in0=lp[:], scalar=0.0,
                    in1=b2r_b[:], op0=mybir.AluOpType.add,
                    op1=mybir.AluOpType.add,
                )
            mx = pc1.tile([128, n_rt], F32)
            nc.vector.tensor_reduce(
                out=mx[:], in_=logits[:], axis=mybir.AxisListType.X,
                op=mybir.AluOpType.max,
            )
            negmx = pc1.tile([128, n_rt], F32)
            nc.vector.tensor_scalar(
                out=negmx[:], in0=mx[:], scalar1=-1.0, scalar2=None,
                op0=mybir.AluOpType.mult,
            )
            sm = pc1.tile([128, n_rt], F32)
            for r in range(n_rt):
                ext = pc.tile([128, C], BF16, tag="ext")
                nc.scalar.activation(
                    out=ext[:], in_=logits[:, r, :],
                    func=mybir.ActivationFunctionType.Exp,
                    bias=negmx[:, r:r + 1],
                    accum_out=sm[:, r:r + 1],
                )
            logz = pc1.tile([128, n_rt], F32)
            nc.scalar.activation(
                out=logz[:], in_=sm[:], func=mybir.ActivationFunctionType.Ln
            )
            mxz = pc1.tile([128, n_rt], F32)
            nc.vector.tensor_tensor(out=mxz[:], in0=mx[:], in1=logz[:],
                                    op=mybir.AluOpType.add)
            for r in range(n_rt):
                rows = 128 if r < n_rt - 1 else rt_tail
                outf = pc.tile([128, C], F32, tag="outf")
                nc.vector.tensor_scalar(
                    out=outf[:rows, :], in0=logits[:rows, r, :],
                    scalar1=mxz[:rows, r:r + 1], scalar2=None,
                    op0=mybir.AluOpType.subtract,
                )
                if r < n_rt - 1:
                    nc.sync.dma_start(
                        out=out_ext[r * 128:(r + 1) * 128, :],
                        in_=outf[:, :],
                    )
                else:
                    nc.sync.dma_start(
                        out=out_ext[r * 128:r * 128 + rows, :],
                        in_=outf[:rows, :],
                    )

    return nc


# ----------------------------------------------------------------------------
# host entry point
# ----------------------------------------------------------------------------
def make_in_maps(cfg, prep, inputs):
    P, n_loc, H, C = cfg.P, cfg.n_loc, cfg.H, cfg.C
    n_rt, n_kc = cfg.n_rt, cfg.n_kc
    x = np.asarray(inputs["x"], dtype=np.float32)
    w1 = np.asarray(inputs["lin1_w"], dtype=np.float32)
    b1 = np.asarray(inputs["lin1_b"], dtype=np.float32)
    w2 = np.asarray(inputs["lin2_w"], dtype=np.float32)
    b2 = np.asarray(inputs["lin2_b"], dtype=np.float32)
    beta1 = np.asarray(inputs["beta1"], dtype=np.float32)
    beta2 = np.asarray(inputs["beta2"], dtype=np.float32)

    # w1t[p, k*H + h] = w1[h, k*128+p]
    w1t = np.ascontiguousarray(
        w1.reshape(H, n_kc, 128).transpose(2, 1, 0).reshape(128, n_kc * H)
    ).astype(NPBF)
    b1r = np.broadcast_to(b1[None, :], (128, H)).copy()
    w2t = np.ascontiguousarray(w2.T).astype(NPBF)
    b2r = np.broadcast_to(b2[None, :], (128, C)).astype(NPBF).copy()
    b1r_t = np.broadcast_to(beta1.reshape(1, 1), (128, 1)).copy()
    b2r_t = np.broadcast_to(beta2.reshape(1, 1), (128, 1)).copy()
    ident = np.eye(128, dtype=np.float32).astype(NPBF)

    in_maps = []
    for c in range(P):
        xs = x[c * n_loc:(c + 1) * n_loc]            # [n_loc, F_in]
        # xTp[p, r*1024 + k*128 + n] = x[c*n_loc + r*128 + n, k*128 + p]
        xp = np.zeros((128, n_rt * n_kc * 128), dtype=NPBF)
        xsb = xs.astype(NPBF)
        for r in range(n_rt):
            rows = min(128, n_loc - r * 128)
            blk = xsb[r * 128:r * 128 + rows].reshape(rows, n_kc, 128)
            xp_r = xp[:, r * 1024:(r + 1) * 1024].reshape(128, n_kc, 128)
            xp_r[:, :, :rows] = blk.transpose(2, 1, 0)
        in_maps.append(
            {
                "xTp": xp,
                "w1t": w1t,
                "b1r": b1r,
                "w2t": w2t,
                "b2r": b2r,
                "beta1r": b1r_t,
                "beta2r": b2r_t,
                "idx_src_w": prep["idx_src_w"][c],
                "m_host": prep["m_host"][c],
                "mt_host": prep["mt_host"][c],
                "ident": ident,
            }
        )
    return in_maps


def run(inputs, trace=False, tmpdir=None, cfg=None):
    from concourse.bass_utils import run_bass_kernel_spmd

    if cfg is None:
        cfg = Cfg()
    prep = host_prep(cfg, np.asarray(inputs["edge_index"]))
    nc = build_program(cfg, prep)
    nc.finalize()
    in_maps = make_in_maps(cfg, prep, inputs)
    res = run_bass_kernel_spmd(
        nc, in_maps, core_ids=list(range(cfg.P)), trace=trace, tmpdir=tmpdir
    )
    outs = [res.results[i]["out"] for i in range(cfg.P)]
    return np.concatenate(outs, axis=0).astype(np.float32), res


def kernel(**inputs) -> np.ndarray:
    out, _ = run(inputs)
    return out


if __name__ == "__main__":
    pass


# revision 23
# speedup vs baseline: 1.0441x; 1.0441x over previous
"""AGNN (2-layer) distributed Bass kernel for one TRN2 chip (8 NeuronCores).

v2 design. Nodes row-block-sharded across 8 cores (6250 each); each core owns
edges whose destination is local. Per AGNN layer, per core:
  - builds a 256B table row per local node: [xn(64) | rcpn | norm | pad] bf16
    (256B is the dma_gather minimum row size)
  - the table is split A/B by local row (A = rows 0:4096, B = 4096:6250) so
    src-gather indices fit int16 after AllGather concatenation, and so the
    pass-A edge processing overlaps the table-B AllGather
  - gathers per-edge SRC rows only (dma_gather, SEG chunks per call).
    DST features are NOT gathered: per 128-edge chunk a small PE matmul
    dst_feat[e,f] = sum_w MT[w,e] * wtab[w,f] materializes them from the
    window's 64 local table rows (wtab, SBUF) using a host-precomputed
    transposed one-hot mask MT streamed via plain contiguous DMA
  - per-edge dots on DVE against the dst_feat PSUM (8 chunks per PSUM bank),
    tree-reduced; w = exp(beta*a); wn = w*norm_src
  - scatter-add via one-hot matmuls: lhsT = M (host-streamed one-hot, scaled
    by wn), rhs = [xn_src | rcpn_src] -> psum [64, 65] per window gives
    numerator and denominator simultaneously
lin1/lin2 GEMMs are node-parallel (x shipped pre-transposed bf16 in
partition-contiguous layout); log_softmax fused at the end.
"""

import contextlib
import numpy as np
import ml_dtypes

from concourse import bacc, bass, mybir, tile

BF16 = mybir.dt.bfloat16
F32 = mybir.dt.float32
I16 = mybir.dt.int16
NPBF = ml_dtypes.bfloat16

EPS = 1e-12
# TimelineSim cost-model total is 1,335,398 ns for this config; the same
# simulator scores the previous 752,900 ns HW baseline at 1,479,300 ns, so
# the expected HW time is ~1335 * (752.9/1479.3) ~= 680 us.  Engine busy in
# the sim: DMA 650us (gathers 333 + mask streams/repacks), DVE 410us,
# collectives 390us (4x packed-row AllGathers), Pool 340us, PE 186us.
LAST_COST_MODEL_NS = 680_000
ROW = 128          # table row width (bf16 elements) = 256 bytes (HW minimum)
RCPN, NORM = 64, 65   # column layout within a table row


class Cfg:
    def __init__(self, N=50000, E=800000, F_in=1024, H=64, C=256, P=8,
                 WIN=64, CHUNK=128, WAVE_WINDOWS=6, SEG=8, SPLA=4096,
                 debug_layers=2):
        self.N, self.E, self.F_in, self.H, self.C, self.P = N, E, F_in, H, C, P
        self.WIN, self.CHUNK = WIN, CHUNK
        self.WAVE_WINDOWS = WAVE_WINDOWS
        self.SEG = SEG
        self.debug_layers = debug_layers
        assert N % P == 0
        self.n_loc = N // P
        self.SPLA = SPLA                      # local rows in table A
        self.SPLB = self.n_loc - SPLA         # local rows in table B
        assert SPLA % 128 == 0
        self.n_rtA = SPLA // 128
        assert P * SPLA <= 32768 and P * self.SPLB <= 32768
        self.n_win = (self.n_loc + WIN - 1) // WIN
        self.n_rt = (self.n_loc + 127) // 128
        self.rt_tail = self.n_loc - (self.n_rt - 1) * 128
        assert F_in % 128 == 0
        self.n_kc = F_in // 128


# ----------------------------------------------------------------------------
# host-side edge prep
# ----------------------------------------------------------------------------
def host_prep(cfg, edge_index):
    src = np.asarray(edge_index[0], dtype=np.int64)
    dst = np.asarray(edge_index[1], dtype=np.int64)
    P, n_loc, WIN, CHUNK, n_win = cfg.P, cfg.n_loc, cfg.WIN, cfg.CHUNK, cfg.n_win
    SPLA, SPLB = cfg.SPLA, cfg.SPLB

    core_of = dst // n_loc
    win_of = (dst % n_loc) // WIN
    lr_src = src % n_loc
    core_src = src // n_loc
    b_of = (lr_src >= SPLA).astype(np.int64)
    tab_idx = np.where(b_of, core_src * SPLB + (lr_src - SPLA),
                       core_src * SPLA + lr_src)
    key = (core_of * n_win + win_of) * 2 + b_of
    order = np.argsort(key, kind="stable")
    bounds = np.searchsorted(key[order], np.arange(P * n_win * 2 + 1))

    def bucket_len(c, w, h):
        i = (c * n_win + w) * 2 + h
        return bounds[i + 1] - bounds[i]

    def bucket(c, w, h):
        i = (c * n_win + w) * 2 + h
        return order[bounds[i]:bounds[i + 1]]

    K_A = np.zeros(n_win, dtype=np.int64)
    K_B = np.zeros(n_win, dtype=np.int64)
    for w in range(n_win):
        mA = max(bucket_len(c, w, 0) for c in range(P))
        mB = max(bucket_len(c, w, 1) for c in range(P))
        K_A[w] = max(1, -(-mA // CHUNK))
        K_B[w] = max(1, -(-mB // CHUNK))
    cA_total = int(K_A.sum())
    c_total = cA_total + int(K_B.sum())

    win_of_chunk = np.zeros(c_total, dtype=np.int64)
    start_flag = np.zeros(c_total, dtype=bool)
    stop_flag = np.zeros(c_total, dtype=bool)

    # per-core slot arrays
    slots_tab = np.zeros((P, c_total * 128), dtype=np.int64)   # table row idx
    slots_rel = np.full((P, c_total * 128), -1.0, dtype=np.float32)

    # waves: two sequences (pass A over K_A chunks, pass B over K_B chunks)
    waves = []
    j0 = 0
    for h, K in ((0, K_A), (1, K_B)):
        for w0 in range(0, n_win, cfg.WAVE_WINDOWS):
            ws = list(range(w0, min(w0 + cfg.WAVE_WINDOWS, n_win)))
            Wc = int(sum(K[w] for w in ws))
            col = j0
            for w in ws:
                start_flag[col] = True
                for c in range(P):
                    el = bucket(c, w, h)
                    k = np.arange(len(el))
                    pos = (col + k // CHUNK) * 128 + k % CHUNK
                    slots_tab[c, pos] = tab_idx[el]
                    slots_rel[c, pos] = (dst[el] % n_loc) - w * WIN
                win_of_chunk[col:col + K[w]] = w
                col += int(K[w])
                stop_flag[col - 1] = True
            waves.append(dict(j0=j0, Wc=Wc, n_wins=len(ws), w_base=w0,
                              is_b=h))
            j0 += Wc
    assert j0 == c_total
    n_waves_a = sum(1 for v in waves if v["is_b"] == 0)

    # wrapped int16 idx layout (position-uniform: slot i -> col i//16 with
    # 8 replicated 16-partition groups)
    idx_src_w = np.zeros((P, 128, c_total * 8), dtype=np.int16)
    i = np.arange(c_total * 128)
    cols = i // 16
    rows = i % 16
    for c in range(P):
        v = slots_tab[c].astype(np.int16)
        for g in range(8):
            idx_src_w[c, g * 16 + rows, cols] = v

    # masks, bf16: M per-wave w-major [128, WIN*Wc] blocks (so the wn scale
    # multiply has packed innermost dims -> DVE 2x); MT [WIN, c_total*128]
    rel = slots_rel.reshape(P, c_total, 128)
    iota = np.arange(WIN, dtype=np.float32)
    m_host = np.zeros((P, 128, c_total * WIN), dtype=NPBF)
    mt_host = np.zeros((P, WIN, c_total * 128), dtype=NPBF)
    for c in range(P):
        mm = (rel[c][:, :, None] == iota[None, None, :])      # [ct, 128, WIN]
        mt_host[c] = np.ascontiguousarray(
            mm.transpose(2, 0, 1).reshape(WIN, c_total * 128)).astype(NPBF)
        for v in waves:
            j0, Wc = v["j0"], v["Wc"]
            blk = mm[j0:j0 + Wc].transpose(1, 2, 0)           # [128, WIN, Wc]
            m_host[c][:, j0 * WIN:(j0 + Wc) * WIN] = np.ascontiguousarray(
                blk.reshape(128, WIN * Wc)).astype(NPBF)

    wc_max = max(v["Wc"] for v in waves)
    return dict(
        idx_src_w=idx_src_w,
        m_host=m_host,
        mt_host=mt_host,
        waves=waves,
        n_waves_a=n_waves_a,
        c_total=c_total,
        cA_total=cA_total,
        wc_max=wc_max,
        win_of_chunk=win_of_chunk,
        start_flag=start_flag,
        stop_flag=stop_flag,
        dbg_slots_tab=slots_tab,
        dbg_slots_rel=slots_rel,
    )


# ----------------------------------------------------------------------------
# numpy emulator of the device algorithm (host-prep validation)
# ----------------------------------------------------------------------------
def emulate(cfg, prep, inputs):
    x = np.asarray(inputs["x"], dtype=np.float64)
    w1 = np.asarray(inputs["lin1_w"], dtype=np.float64)
    b1 = np.asarray(inputs["lin1_b"], dtype=np.float64)
    w2 = np.asarray(inputs["lin2_w"], dtype=np.float64)
    b2 = np.asarray(inputs["lin2_b"], dtype=np.float64)
    betas = [float(inputs["beta1"][0]), float(inputs["beta2"][0])]
    P, n_loc, WIN = cfg.P, cfg.n_loc, cfg.WIN
    SPLA, SPLB = cfg.SPLA, cfg.SPLB
    c_total, cA = prep["c_total"], prep["cA_total"]
    win_of_chunk = prep["win_of_chunk"]
    slots_tab = prep["dbg_slots_tab"]
    slots_rel = prep["dbg_slots_rel"]

    h = np.maximum(x @ w1.T + b1, 0.0)
    for layer in range(cfg.debug_layers):
        beta = betas[layer]
        nrm = np.maximum(np.linalg.norm(h, axis=1), EPS)
        xn = h / nrm[:, None]
        h2 = np.zeros_like(h)
        for c in range(P):
            num = np.zeros((n_loc, cfg.H))
            den = np.zeros(n_loc)
            for j in range(c_total):
                is_b = j >= cA
                w = win_of_chunk[j]
                sl = slice(j * 128, (j + 1) * 128)
                rel = slots_rel[c, sl]
                valid = rel >= 0
                ti = slots_tab[c, sl][valid]
                if is_b:
                    s_node = (ti // SPLB) * n_loc + SPLA + ti % SPLB
                else:
                    s_node = (ti // SPLA) * n_loc + ti % SPLA
                d_loc = (w * WIN + rel[valid]).astype(np.int64)
                a = np.sum(xn[s_node] * xn[c * n_loc + d_loc], axis=1)
                wgt = np.exp(beta * a)
                np.add.at(num, d_loc, wgt[:, None] * h[s_node])
                np.add.at(den, d_loc, wgt)
            h2[c * n_loc:(c + 1) * n_loc] = num / np.maximum(den, EPS)[:, None]
        h = h2
    logits = h @ w2.T + b2
    lse = np.log(np.sum(np.exp(logits - logits.max(axis=1, keepdims=True)),
                        axis=1)) + logits.max(axis=1)
    return logits - lse[:, None]


# ----------------------------------------------------------------------------
# device program
# ----------------------------------------------------------------------------
def build_program(cfg, prep):
    P, H, C, F_in = cfg.P, cfg.H, cfg.C, cfg.F_in
    n_loc, n_rt, rt_tail, n_kc = cfg.n_loc, cfg.n_rt, cfg.rt_tail, cfg.n_kc
    WIN, N, SPLA, SPLB = cfg.WIN, cfg.N, cfg.SPLA, cfg.SPLB
    n_rtA, SEG = cfg.n_rtA, cfg.SEG
    c_total, waves, wc_max = prep["c_total"], prep["waves"], prep["wc_max"]
    win_of_chunk = prep["win_of_chunk"]
    start_flag, stop_flag = prep["start_flag"], prep["stop_flag"]
    n_win = cfg.n_win

    nc = bacc.Bacc("TRN2", target_bir_lowering=False,
                   dynamic_dma_scratch_size=16 * 128 * SEG)

    xTp_ext = nc.declare_dram_parameter("xTp", [128, n_rt * 8 * 128], BF16,
                                        isOutput=False)
    w1t_ext = nc.declare_dram_parameter("w1t", [128, n_kc * H], BF16,
                                        isOutput=False)
    b1r_ext = nc.declare_dram_parameter("b1r", [128, H], F32, isOutput=False)
    w2t_ext = nc.declare_dram_parameter("w2t", [H, C], BF16, isOutput=False)
    b2r_ext = nc.declare_dram_parameter("b2r", [128, C], BF16, isOutput=False)
    beta1_ext = nc.declare_dram_parameter("beta1r", [128, 1], F32, isOutput=False)
    beta2_ext = nc.declare_dram_parameter("beta2r", [128, 1], F32, isOutput=False)
    isrc_ext = nc.declare_dram_parameter(
        "idx_src_w", [128, c_total * 8], I16, isOutput=False)
    m_ext = nc.declare_dram_parameter(
        "m_host", [128, c_total * WIN], BF16, isOutput=False)
    mt_ext = nc.declare_dram_parameter(
        "mt_host", [WIN, c_total * 128], BF16, isOutput=False)
    ident_ext = nc.declare_dram_parameter("ident", [128, 128], BF16,
                                          isOutput=False)
    out_ext = nc.declare_dram_parameter("out", [n_loc, C], F32, isOutput=True)

    PK = NORM + 1   # packed row width (132B) for collective transport
    loc_packA = [nc.dram_tensor(f"loc_packA{i}", [SPLA, PK], BF16) for i in (0, 1)]
    loc_packB = [nc.dram_tensor(f"loc_packB{i}", [SPLB, PK], BF16) for i in (0, 1)]
    full_packA = [
        nc.dram_tensor(f"full_packA{i}", [P * SPLA, PK], BF16, addr_space="Shared")
        for i in (0, 1)
    ]
    full_packB = [
        nc.dram_tensor(f"full_packB{i}", [P * SPLB, PK], BF16, addr_space="Shared")
        for i in (0, 1)
    ]
    full_tabA = [nc.dram_tensor(f"full_tabA{i}", [P * SPLA, ROW], BF16)
                 for i in (0, 1)]
    full_tabB = [nc.dram_tensor(f"full_tabB{i}", [P * SPLB, ROW], BF16)
                 for i in (0, 1)]
    replica = [list(range(P))]

    with contextlib.ExitStack() as es:
        tc = es.enter_context(tile.TileContext(nc))
        const = es.enter_context(tc.tile_pool(name="const", bufs=1))
        sb = es.enter_context(tc.tile_pool(name="sb", bufs=1))

        # ------- constants -------
        w1t_b = const.tile([128, n_kc, H], BF16)
        nc.sync.dma_start(out=w1t_b[:], in_=w1t_ext[:].rearrange(
            "p (k h) -> p k h", k=n_kc))
        b1r = const.tile([128, H], F32)
        nc.sync.dma_start(out=b1r[:], in_=b1r_ext[:])
        b2r_b = const.tile([128, C], BF16)
        nc.sync.dma_start(out=b2r_b[:], in_=b2r_ext[:])
        w2t_b = const.tile([H, C], BF16)
        nc.sync.dma_start(out=w2t_b[:], in_=w2t_ext[:])
        beta = []
        for i, ext in enumerate((beta1_ext, beta2_ext)):
            bt = const.tile([128, 1], F32, name=f"beta{i}")
            nc.sync.dma_start(out=bt[:], in_=ext[:])
            beta.append(bt)
        ident = const.tile([128, 128], BF16)
        nc.sync.dma_start(out=ident[:], in_=ident_ext[:])
        isrc = const.tile([128, c_total * 8], I16)
        nc.sync.dma_start(out=isrc[:], in_=isrc_ext[:])

        h_loc = sb.tile([128, n_rt, H], BF16)
        nsq = sb.tile([128, n_rt], F32)
        nc.vector.memset(h_loc[:, n_rt - 1, :], 0.0)

        # ------- phase A: lin1 + relu (+ per-tile nsq) -------
        with tc.tile_pool(name="ph_a", bufs=3) as pa, \
             tc.tile_pool(name="ph_a_ps", bufs=2, space="PSUM") as pa_ps:
            for r in range(n_rt):
                rows = 128 if r < n_rt - 1 else rt_tail
                xt_b = pa.tile([128, n_kc, 128], BF16, tag="xt_b")
                nc.sync.dma_start(
                    out=xt_b[:],
                    in_=xTp_ext[:, r * 1024:(r + 1) * 1024].rearrange(
                        "p (k n) -> p k n", k=n_kc),
                )
                hp = pa_ps.tile([128, H], F32, tag="hp")
                for k in range(n_kc):
                    nc.tensor.matmul(
                        hp[:rows, :],
                        lhsT=xt_b[:, k, :rows],
                        rhs=w1t_b[:, k, :],
                        start=(k == 0),
                        stop=(k == n_kc - 1),
                    )
                hb = pa.tile([128, H], F32, tag="hb")
                nc.vector.tensor_tensor(
                    out=hb[:rows, :], in0=hp[:rows, :], in1=b1r[:rows, :],
                    op=mybir.AluOpType.add,
                )
                nc.vector.tensor_scalar(
                    out=h_loc[:rows, r, :], in0=hb[:rows, :],
                    scalar1=0.0, scalar2=None, op0=mybir.AluOpType.max,
                )
                sq = pa.tile([128, H], BF16, tag="sq")
                nc.scalar.activation(
                    out=sq[:], in_=h_loc[:, r, :],
                    func=mybir.ActivationFunctionType.Square,
                    accum_out=nsq[:, r:r + 1],
                )

        # ------- shared per-layer tiles -------
        tabrow = sb.tile([128, n_rt, ROW], BF16)
        wtab = sb.tile([WIN, n_win, WIN], BF16)
        aggA = sb.tile([128, n_rt, H], BF16)
        aggB = sb.tile([128, n_rt, H], BF16)
        denA = sb.tile([128, n_rt], F32)
        denB = sb.tile([128, n_rt], F32)

        # ------- table build + allgather (split A/B by local row) -------
        def build_tables(src_tile, nsq_tile, layer):
            norm = sb.tile([128, n_rt], F32, name=f"norm{layer}")
            rcpn = sb.tile([128, n_rt], F32, name=f"rcpn{layer}")
            nc.scalar.sqrt(norm[:], nsq_tile[:])
            nc.vector.tensor_scalar(
                out=norm[:], in0=norm[:], scalar1=float(EPS), scalar2=None,
                op0=mybir.AluOpType.max,
            )
            nc.vector.reciprocal(rcpn[:], norm[:])

            def fill_rows(r0, r1):
                nc.vector.memset(tabrow[:, r0:r1, :], 0.0)
                for r in range(r0, r1):
                    nc.vector.tensor_scalar(
                        out=tabrow[:, r, 0:H], in0=src_tile[:, r, :],
                        scalar1=rcpn[:, r:r + 1], scalar2=None,
                        op0=mybir.AluOpType.mult,
                    )
                nc.vector.tensor_copy(tabrow[:, r0:r1, RCPN], rcpn[:, r0:r1])
                nc.vector.tensor_copy(tabrow[:, r0:r1, NORM], norm[:, r0:r1])

            # part A: r-tiles [0, n_rtA).  loc writes ride the gpsimd (Pool)
            # queue so their waits never block the SP stream queue; repacks
            # are emitted inside agnn_layer just before the pass that needs
            # them (again on gpsimd, ahead of that pass's gathers).
            fill_rows(0, n_rtA)
            ltA = loc_packA[layer]
            nc.sync.dma_start(
                out=ltA[:].rearrange("(t p) f -> p t f", p=128),
                in_=tabrow[:, 0:n_rtA, 0:PK],
            )
            nc.gpsimd.collective_compute(
                "AllGather", mybir.AluOpType.bypass, replica_groups=replica,
                ins=[ltA[:].opt()], outs=[full_packA[layer][:].opt()],
            )
            # part B: r-tiles [n_rtA, n_rt)
            fill_rows(n_rtA, n_rt)
            ltB = loc_packB[layer]
            nfullB = (n_rt - 1 - n_rtA) * 128
            if nfullB > 0:
                nc.sync.dma_start(
                    out=ltB[0:nfullB, :].rearrange("(t p) f -> p t f", p=128),
                    in_=tabrow[:, n_rtA:n_rt - 1, 0:PK],
                )
            nc.sync.dma_start(
                out=ltB[nfullB:SPLB, :], in_=tabrow[0:rt_tail, n_rt - 1, 0:PK]
            )
            nc.gpsimd.collective_compute(
                "AllGather", mybir.AluOpType.bypass, replica_groups=replica,
                ins=[ltB[:].opt()], outs=[full_packB[layer][:].opt()],
            )
            # window-major dst table (xn only): even/odd windows
            nc.sync.dma_start(out=wtab[:, 0:n_win:2, :],
                              in_=tabrow[0:WIN, :, 0:WIN])
            nc.sync.dma_start(out=wtab[:, 1:n_win:2, :],
                              in_=tabrow[WIN:128, :, 0:WIN])

        # ------- one AGNN layer -------
        def agnn_layer(layer, beta_tile, agg_out, nsq_out, work, psum_s,
                       psum_d):
            for agg_t, den_t, is_b in ((aggA, denA, 0), (aggB, denB, 1)):
                ft = (full_tabB if is_b else full_tabA)[layer]
                fp = (full_packB if is_b else full_packA)[layer]
                # repack 132B collective rows to the 256B-stride gather table;
                # on the gpsimd queue right ahead of this pass's gathers
                # (halved: DMA APs must stay under 32768 descriptors)
                nrows = fp.shape[0]
                nc.scalar.dma_start(out=ft[0:nrows // 2, 0:PK],
                                    in_=fp[0:nrows // 2, :])
                nc.scalar.dma_start(out=ft[nrows // 2:nrows, 0:PK],
                                    in_=fp[nrows // 2:nrows, :])
                for g, wave in enumerate(waves):
                    if wave["is_b"] != is_b:
                        continue
                    Wc, j0, w_base = wave["Wc"], wave["j0"], wave["w_base"]
                    gsrc = work.tile([128, wc_max, ROW], BF16, tag="gsrc")
                    for s0 in range(0, Wc, SEG):
                        sn = min(SEG, Wc - s0)
                        nc.gpsimd.dma_gather(
                            out_ap=gsrc[:, s0:s0 + sn, :],
                            in_ap=ft[:, :],
                            idxs_ap=isrc[:, (j0 + s0) * 8:(j0 + s0 + sn) * 8],
                            num_idxs=sn * 128, num_idxs_reg=sn * 128,
                            elem_size=ROW,
                        )
                    mt_t = work.tile([WIN, wc_max, 128], BF16, tag="mt")
                    nc.sync.dma_start(
                        out=mt_t[:, 0:Wc, :],
                        in_=mt_ext[:, j0 * 128:(j0 + Wc) * 128].rearrange(
                            "w (c e) -> w c e", c=Wc),
                    )
                    m_t = work.tile([128, wc_max * WIN], BF16, tag="m")
                    nc.sync.dma_start(
                        out=m_t[:, 0:Wc * WIN],
                        in_=m_ext[:, j0 * WIN:(j0 + Wc) * WIN],
                    )
                    dotb = work.tile([128, wc_max, WIN], BF16, tag="dot")
                    for g0 in range(0, Wc, 8):
                        gn = min(8, Wc - g0)
                        psd = psum_d.tile([128, 8, WIN], F32, tag="psd",
                                          name=f"psd_{layer}_{g}_{g0}")
                        for k in range(gn):
                            j = j0 + g0 + k
                            nc.tensor.matmul(
                                psd[:, k, :],
                                lhsT=mt_t[:, g0 + k, :],
                                rhs=wtab[:, int(win_of_chunk[j]), :],
                                start=True, stop=True,
                            )
                        nc.vector.tensor_tensor(
                            out=dotb[:, g0:g0 + gn, :],
                            in0=gsrc[:, g0:g0 + gn, 0:H],
                            in1=psd[:, 0:gn, :],
                            op=mybir.AluOpType.mult,
                        )
                    width = WIN
                    while width > 2:
                        half = width // 2
                        nc.vector.tensor_tensor(
                            out=dotb[:, 0:Wc, 0:half],
                            in0=dotb[:, 0:Wc, 0:half],
                            in1=dotb[:, 0:Wc, half:width],
                            op=mybir.AluOpType.add,
                        )
                        width = half
                    a_t = work.tile([128, wc_max], BF16, tag="a_t")
                    nc.vector.tensor_tensor(
                        out=a_t[:, 0:Wc], in0=dotb[:, 0:Wc, 0],
                        in1=dotb[:, 0:Wc, 1], op=mybir.AluOpType.add,
                    )
                    w_t = work.tile([128, wc_max], BF16, tag="w_t")
                    nc.scalar.activation(
                        out=w_t[:, 0:Wc], in_=a_t[:, 0:Wc],
                        func=mybir.ActivationFunctionType.Exp,
                        scale=beta_tile[:, 0:1],
                    )
                    wn_t = work.tile([128, wc_max], BF16, tag="wn_t")
                    nc.vector.tensor_tensor(
                        out=wn_t[:, 0:Wc], in0=w_t[:, 0:Wc],
                        in1=gsrc[:, 0:Wc, NORM], op=mybir.AluOpType.mult,
                    )
                    # m is w-major per wave: [128, WIN, Wc]; wn broadcasts
                    # along w with packed innermost c -> DVE 2x mode
                    m_v = m_t[:, 0:Wc * WIN].rearrange("p (w c) -> p w c", c=Wc)
                    nc.vector.tensor_tensor(
                        out=m_v, in0=m_v,
                        in1=wn_t[:, None, 0:Wc].broadcast_to([128, WIN, Wc]),
                        op=mybir.AluOpType.mult,
                    )
                    pslots = [
                        psum_s.tile([WIN, H + 1], F32, tag=f"ps{s}",
                                    name=f"ps{s}_{layer}_{g}")
                        for s in range(wave["n_wins"])
                    ]
                    for k in range(Wc):
                        j = j0 + k
                        s = int(win_of_chunk[j] - w_base)
                        nc.tensor.matmul(
                            pslots[s][:, :],
                            lhsT=m_t[:, k:k + (WIN - 1) * Wc + 1:Wc],
                            rhs=gsrc[:, k, 0:H + 1],
                            start=bool(start_flag[j]),
                            stop=bool(stop_flag[j]),
                        )
                    for s in range(wave["n_wins"]):
                        w = w_base + s
                        prow = (w % 2) * WIN
                        nc.scalar.copy(
                            agg_t[prow:prow + WIN, w // 2, :],
                            pslots[s][:, 0:H],
                        )
                        nc.vector.tensor_copy(
                            den_t[prow:prow + WIN, w // 2:w // 2 + 1],
                            pslots[s][:, H:H + 1],
                        )
            # combine A+B, divide
            den_r = sb.tile([128, n_rt], F32, name=f"den{layer}")
            nc.vector.tensor_tensor(out=den_r[:], in0=denA[:], in1=denB[:],
                                    op=mybir.AluOpType.add)
            nc.vector.tensor_scalar(
                out=den_r[:], in0=den_r[:], scalar1=float(EPS), scalar2=None,
                op0=mybir.AluOpType.max,
            )
            rcpden = sb.tile([128, n_rt], F32, name=f"rcpden{layer}")
            nc.vector.reciprocal(rcpden[:], den_r[:])
            rcpden_b = sb.tile([128, n_rt], BF16, name=f"rcpdenb{layer}")
            nc.vector.tensor_copy(rcpden_b[:], rcpden[:])
            num = sb.tile([128, n_rt, H], BF16, name=f"num{layer}")
            nc.vector.tensor_tensor(out=num[:], in0=aggA[:], in1=aggB[:],
                                    op=mybir.AluOpType.add)
            nc.vector.tensor_tensor(
                out=agg_out[:, :, :], in0=num[:, :, :],
                in1=rcpden_b[:, :].to_broadcast([128, n_rt, H]),
                op=mybir.AluOpType.mult,
            )
            sqscr = sb.tile([128, n_rt, H], BF16, name=f"sq{layer}")
            nc.vector.tensor_tensor(
                out=sqscr[:], in0=agg_out[:], in1=agg_out[:],
                op=mybir.AluOpType.mult,
            )
            nc.vector.tensor_reduce(
                out=nsq_out[:], in_=sqscr[:], axis=mybir.AxisListType.X,
                op=mybir.AluOpType.add,
            )

        x1 = sb.tile([128, n_rt, H], BF16)
        nsq1 = sb.tile([128, n_rt], F32)
        x2 = sb.tile([128, n_rt, H], BF16)
        nsq2 = sb.tile([128, n_rt], F32)
        with tc.tile_pool(name="work", bufs=2) as work, \
             tc.tile_pool(name="psum_s", bufs=1, space="PSUM") as psum_s, \
             tc.tile_pool(name="psum_d", bufs=2, space="PSUM") as psum_d:
            if cfg.debug_layers >= 1:
                build_tables(h_loc, nsq, layer=0)
                agnn_layer(0, beta[0], x1, nsq1, work, psum_s, psum_d)
            else:
                nc.vector.memset(x1[:], 0.0)
                nc.vector.memset(nsq1[:], 1.0)
            if cfg.debug_layers >= 2:
                build_tables(x1, nsq1, layer=1)
                agnn_layer(1, beta[1], x2, nsq2, work, psum_s, psum_d)
            else:
                nc.vector.memset(x2[:], 0.0)

        # ------- phase C: lin2 + log_softmax -------
        with tc.tile_pool(name="ph_c", bufs=2) as pc, \
             tc.tile_pool(name="ph_c1", bufs=1) as pc1, \
             tc.tile_pool(name="ph_c_ps", bufs=2, space="PSUM") as pc_ps:
            logits = pc1.tile([128, n_rt, C], BF16)
            for r in range(n_rt):
                tp = pc_ps.tile([H, 128], BF16, tag="tp")
                nc.tensor.transpose(tp[:, :], x2[:, r, :], ident[:])
                x2t = pc.tile([H, 128], BF16, tag="x2t")
                nc.scalar.copy(x2t[:], tp[:])
                lp = pc_ps.tile([128, C], F32, tag="lp")
                nc.tensor.matmul(lp[:], lhsT=x2t[:], rhs=w2t_b[:],
                                 start=True, stop=True)
                nc.vector.scalar_tensor_tensor(
                    out=logits[:, r, :], in0=lp[:], scalar=0.0,
                    in1=b2r_b[:], op0=mybir.AluOpType.add,
                    op1=mybir.AluOpType.add,
                )
            mx = pc1.tile([128, n_rt], F32)
            nc.vector.tensor_reduce(
                out=mx[:], in_=logits[:], axis=mybir.AxisListType.X,
                op=mybir.AluOpType.max,
            )
            negmx = pc1.tile([128, n_rt], F32)
            nc.vector.tensor_scalar(
                out=negmx[:], in0=mx[:], scalar1=-1.0, scalar2=None,
                op0=mybir.AluOpType.mult,
            )
            sm = pc1.tile([128, n_rt], F32)
            for r in range(n_rt):
                ext = pc.tile([128, C], BF16, tag="ext")
                nc.scalar.activation(
                    out=ext[:], in_=logits[:, r, :],
                    func=mybir.ActivationFunctionType.Exp,
                    bias=negmx[:, r:r + 1],
                    accum_out=sm[:, r:r + 1],
                )
            logz = pc1.tile([128, n_rt], F32)
            nc.scalar.activation(
                out=logz[:], in_=sm[:], func=mybir.ActivationFunctionType.Ln
            )
            mxz = pc1.tile([128, n_rt], F32)
            nc.vector.tensor_tensor(out=mxz[:], in0=mx[:], in1=logz[:],
                                    op=mybir.AluOpType.add)
            for r in range(n_rt):
                rows = 128 if r < n_rt - 1 else rt_tail
                outf = pc.tile([128, C], F32, tag="outf")
                nc.vector.tensor_scalar(
                    out=outf[:rows, :], in0=logits[:rows, r, :],
                    scalar1=mxz[:rows, r:r + 1], scalar2=None,
                    op0=mybir.AluOpType.subtract,
                )
                if r < n_rt - 1:
                    nc.sync.dma_start(
                        out=out_ext[r * 128:(r + 1) * 128, :],
                        in_=outf[:, :],
                    )
                else:
                    nc.sync.dma_start(
                        out=out_ext[r * 128:r * 128 + rows, :],
                        in_=outf[:rows, :],
                    )

    return nc


# ----------------------------------------------------------------------------
# host entry point
# ----------------------------------------------------------------------------
def make_in_maps(cfg, prep, inputs):
    P, n_loc, H, C = cfg.P, cfg.n_loc, cfg.H, cfg.C
    n_rt, n_kc = cfg.n_rt, cfg.n_kc
    x = np.asarray(inputs["x"], dtype=np.float32)
    w1 = np.asarray(inputs["lin1_w"], dtype=np.float32)
    b1 = np.asarray(inputs["lin1_b"], dtype=np.float32)
    w2 = np.asarray(inputs["lin2_w"], dtype=np.float32)
    b2 = np.asarray(inputs["lin2_b"], dtype=np.float32)
    beta1 = np.asarray(inputs["beta1"], dtype=np.float32)
    beta2 = np.asarray(inputs["beta2"], dtype=np.float32)

    # w1t[p, k*H + h] = w1[h, k*128+p]
    w1t = np.ascontiguousarray(
        w1.reshape(H, n_kc, 128).transpose(2, 1, 0).reshape(128, n_kc * H)
    ).astype(NPBF)
    b1r = np.broadcast_to(b1[None, :], (128, H)).copy()
    w2t = np.ascontiguousarray(w2.T).astype(NPBF)
    b2r = np.broadcast_to(b2[None, :], (128, C)).astype(NPBF).copy()
    b1r_t = np.broadcast_to(beta1.reshape(1, 1), (128, 1)).copy()
    b2r_t = np.broadcast_to(beta2.reshape(1, 1), (128, 1)).copy()
    ident = np.eye(128, dtype=np.float32).astype(NPBF)

    in_maps = []
    for c in range(P):
        xs = x[c * n_loc:(c + 1) * n_loc]            # [n_loc, F_in]
        # xTp[p, r*1024 + k*128 + n] = x[c*n_loc + r*128 + n, k*128 + p]
        xp = np.zeros((128, n_rt * n_kc * 128), dtype=NPBF)
        xsb = xs.astype(NPBF)
        for r in range(n_rt):
            rows = min(128, n_loc - r * 128)
            blk = xsb[r * 128:r * 128 + rows].reshape(rows, n_kc, 128)
            xp_r = xp[:, r * 1024:(r + 1) * 1024].reshape(128, n_kc, 128)
            xp_r[:, :, :rows] = blk.transpose(2, 1, 0)
        in_maps.append(
            {
                "xTp": xp,
                "w1t": w1t,
                "b1r": b1r,
                "w2t": w2t,
                "b2r": b2r,
                "beta1r": b1r_t,
                "beta2r": b2r_t,
                "idx_src_w": prep["idx_src_w"][c],
                "m_host": prep["m_host"][c],
                "mt_host": prep["mt_host"][c],
                "ident": ident,
            }
        )
    return in_maps


def run(inputs, trace=False, tmpdir=None, cfg=None):
    from concourse.bass_utils import run_bass_kernel_spmd

    if cfg is None:
        cfg = Cfg()
    prep = host_prep(cfg, np.asarray(inputs["edge_index"]))
    nc = build_program(cfg, prep)
    nc.finalize()
    in_maps = make_in_maps(cfg, prep, inputs)
    res = run_bass_kernel_spmd(
        nc, in_maps, core_ids=list(range(cfg.P)), trace=trace, tmpdir=tmpdir
    )
    outs = [res.results[i]["out"] for i in range(cfg.P)]
    return np.concatenate(outs, axis=0).astype(np.float32), res


def kernel(**inputs) -> np.ndarray:
    out, _ = run(inputs)
    return out


if __name__ == "__main__":
    pass
